# revision 1
# baseline (speedup 1.0000x reference)
"""Trainium2 Bass kernel for nn_Alignment_vector (cross-attention alignment).

Data-parallel over batch across 8 NeuronCores (4 batches each).

Key structure (v2):
- context transposed via SBUF->SBUF x-bar DMA (no DRAM staging roundtrip)
- MM2 (wcontext = softmax-weights @ context) runs in fp8e4 with DoubleRow
  perf mode (2 k-tiles per matmul at 0.5 cycles/row): E is written fp8
  directly by the Exp activation; context is DMA-loaded a second time as
  fp8 (DMA does the cast).
- Exp uses a constant -4 bias so fp8e4 never overflows (logits <= 10,
  e^6 = 403 < 448); the scale cancels in the following l2_normalize.
- All activation funcs (Copy/Square/Exp/Ln) live in one ACT table
  (natural_log_exp_and_others) so only one table load is emitted.
  1/sqrt chains are computed as Exp(-0.5*Ln(x)).
- Elementwise work is spread: ACT does lrelu-scale/square-accum/exp,
  DVE does max/ttr/sub/mul in bf16 fast modes, Pool does PSUM
  evacuations (qT packs, wc) plus the SWDGE loads.

Math note: the softmax denominator cancels inside the following
l2_normalize, so softmax is computed as a bare (biased) exp().
"""

import math

import numpy as np

import concourse.bacc as bacc
import concourse.tile as tile
import concourse.mybir as mybir
from concourse.masks import make_identity
from concourse.bass_utils import run_bass_kernel_spmd

f32 = mybir.dt.float32
i32 = mybir.dt.int32
bf16 = mybir.dt.bfloat16
fp8 = mybir.dt.float8e4
AF = mybir.ActivationFunctionType
ALU = mybir.AluOpType
PM = mybir.MatmulPerfMode

B, NCORES = 32, 8
BPC = B // NCORES            # batches per core
LQ, LS, D, S = 512, 1024, 1024, 256
NQ, NS, ND = LQ // 128, LS // 128, D // 128   # 4, 8, 8
EXP_BIAS = -4.0              # exp(x - 4); cancels in l2_normalize

LAST_EXEC_TIME_NS = None


def _build(smooth: float):
    nc = bacc.Bacc("TRN2", target_bir_lowering=False, debug=False)

    q_d = nc.dram_tensor("query", (BPC, LQ, D), f32, kind="ExternalInput").ap()
    c_d = nc.dram_tensor("context", (BPC, LS, D), f32, kind="ExternalInput").ap()
    m_d = nc.dram_tensor("matrix", (BPC, LQ, D), f32, kind="ExternalInput").ap()
    W_d = nc.dram_tensor("W", (S, D), f32, kind="ExternalInput").ap()
    bias_d = nc.dram_tensor("b", (S,), f32, kind="ExternalInput").ap()
    out_d = nc.dram_tensor("out", (BPC, LQ, S), f32, kind="ExternalOutput").ap()

    W_s = nc.dram_tensor("W_s", (S, D), bf16, kind="Internal").ap()

    with tile.TileContext(nc) as tc:
        from contextlib import ExitStack
        with ExitStack() as ctx:
            p = lambda *a, **k: ctx.enter_context(tc.tile_pool(*a, **k))
            qf_pool = p(name="qf", bufs=2)
            mf_pool = p(name="mf", bufs=2)
            qm_pool = p(name="qm", bufs=1)
            qT_pool = p(name="qT", bufs=2)
            cxb_pool = p(name="cxb", bufs=1)
            cx8_pool = p(name="cx8", bufs=2)
            cT_pool = p(name="cT", bufs=2)
            al_pool = p(name="al", bufs=1)
            ee_pool = p(name="ee", bufs=2)
            wc_pool = p(name="wc", bufs=2)
            sim_pool = p(name="sim", bufs=1)
            simT_pool = p(name="simT", bufs=1)
            wrk_pool = p(name="wrk", bufs=2)
            t2_pool = p(name="t2", bufs=1)
            sm_pool = p(name="sm", bufs=2)
            out_pool = p(name="outp", bufs=4)
            const_pool = p(name="const", bufs=1)
            psA_pool = p(name="psA", bufs=2, space="PSUM")
            psW_pool = p(name="psW", bufs=2, space="PSUM")
            psT_pool = p(name="psT", bufs=2, space="PSUM")
            psO_pool = p(name="psO", bufs=1, space="PSUM")

            ident = const_pool.tile([128, 128], bf16)
            WT = const_pool.tile([128, ND, S], bf16)
            ones_b = const_pool.tile([1, 128], bf16)
            b_sb = const_pool.tile([1, S], bf16)
            bLNS = const_pool.tile([128, 1], f32)
            bE4 = const_pool.tile([128, 1], f32)

            def rsqrt_dve(dst, srcap, n, scale=None):
                """dst = scale/sqrt(srcap), pure-DVE quake seed + 2 Newton
                steps (avoids ACT table swaps from Ln/Sqrt)."""
                yi = sm_pool.tile([128, n], i32, tag=f"rsq_i{n}")
                t = sm_pool.tile([128, n], f32, tag=f"rsq_t{n}")
                nc.vector.tensor_scalar(out=yi[:], in0=srcap.bitcast(i32),
                                        scalar1=1, scalar2=None,
                                        op0=ALU.arith_shift_right)
                nc.vector.tensor_scalar(out=yi[:], in0=yi[:], scalar1=-1,
                                        scalar2=0x5f3759df, op0=ALU.mult,
                                        op1=ALU.add)
                y = yi[:].bitcast(f32)
                for it in range(2):
                    nc.vector.tensor_tensor(out=t[:], in0=y, in1=y,
                                            op=ALU.mult)
                    nc.vector.tensor_tensor(out=t[:], in0=t[:], in1=srcap,
                                            op=ALU.mult)
                    nc.vector.tensor_scalar(out=t[:], in0=t[:], scalar1=-0.5,
                                            scalar2=1.5, op0=ALU.mult,
                                            op1=ALU.add)
                    last = it == 1 and scale is None
                    nc.vector.tensor_tensor(out=dst if last else yi[:].bitcast(f32),
                                            in0=y, in1=t[:], op=ALU.mult)
                if scale is not None:
                    nc.vector.tensor_scalar_mul(dst, yi[:].bitcast(f32),
                                                scale)

            def const_setup():
                make_identity(nc, ident[:])
                nc.vector.memset(bLNS[:], math.log(float(smooth)))
                nc.vector.memset(bE4[:], EXP_BIAS)
                nc.gpsimd.dma_start(W_s, W_d)
                nc.sync.dma_start(WT[:], W_s, transpose=True)
                nc.vector.memset(ones_b[:], 1.0)
                nc.gpsimd.dma_start(b_sb[:],
                                    bias_d.rearrange("(o s) -> o s", o=1))

            def stage_l(bi):
                """DMA loads (Pool SWDGE) + context x-bar transposes (SP).
                cxb goes first: cxb -> xbar -> cT gates MM1."""
                cxb = cxb_pool.tile([128, NS, D], bf16)
                nc.gpsimd.dma_start(
                    cxb[:], c_d[bi].rearrange("(t p) d -> p t d", p=128))
                cT = cT_pool.tile([128, ND, LS], bf16)
                for t in range(NS):
                    nc.sync.dma_start(cT[:, :, 128 * t:128 * (t + 1)],
                                      cxb[:, t, :], transpose=True)
                qf = qf_pool.tile([128, NQ, D], bf16)
                nc.gpsimd.dma_start(
                    qf[:], q_d[bi].rearrange("(t p) d -> p t d", p=128))
                mf = mf_pool.tile([128, NQ, D], bf16)
                nc.gpsimd.dma_start(
                    mf[:], m_d[bi].rearrange("(t p) d -> p t d", p=128))
                cx8 = cx8_pool.tile([128, NS, D], fp8)
                nc.gpsimd.dma_start(
                    cx8[:], c_d[bi].rearrange("(t p) d -> p t d", p=128))
                return dict(qf=qf, mf=mf, cx8=cx8, cT=cT)

            def stage_x1a(bi, t):
                """qm product + PE transpose into qT (Pool evacuates)."""
                qf, mf = t["qf"], t["mf"]
                qm = qm_pool.tile([128, NQ, D], bf16)
                nc.vector.tensor_tensor(out=qm[:], in0=qf[:], in1=mf[:],
                                    op=ALU.mult)
                qT = qT_pool.tile([128, ND, LQ], bf16)
                for tq in range(NQ):
                    pst = psT_pool.tile([128, ND, 128], bf16, tag="psT")
                    for k in range(ND):
                        nc.tensor.transpose(
                            pst[:, k, :], qm[:, tq, 128 * k:128 * (k + 1)],
                            ident[:])
                    nc.vector.tensor_copy(
                        qT[:, :, 128 * tq:128 * (tq + 1)], pst[:])
                t["qT"] = qT
                return t

            def stage_x1b(bi, t):
                """MM1 + lrelu + row l2-norm + biased exp -> E (fp8)."""
                cT, qT = t["cT"], t["qT"]
                AL = al_pool.tile([128, NS, LQ], bf16)
                ss = sm_pool.tile([128, NS], f32, tag="ss")
                for m in range(NS):
                    psA = psA_pool.tile([128, LQ], f32)
                    for k in range(ND):
                        nc.tensor.matmul(
                            psA[:], lhsT=cT[:, k, 128 * m:128 * (m + 1)],
                            rhs=qT[:, k, :],
                            start=(k == 0), stop=(k == ND - 1))
                    alr = wrk_pool.tile([128, LQ], bf16, tag="alr")
                    nc.scalar.activation(alr[:], psA[:], AF.Copy)
                    t01 = wrk_pool.tile([128, LQ], bf16, tag="t01")
                    nc.vector.tensor_scalar_mul(t01[:], alr[:], 0.1)
                    nc.vector.tensor_tensor(out=AL[:, m, :], in0=alr[:],
                                            in1=t01[:], op=ALU.max)
                    sqd = wrk_pool.tile([128, LQ], bf16, tag="sqd")
                    nc.scalar.activation(sqd[:], AL[:, m, :], AF.Square,
                                         accum_out=ss[:, m:m + 1])
                rs = sm_pool.tile([128, NS], f32, tag="rs")
                rsqrt_dve(rs[:], ss[:], NS, scale=float(smooth))
                E = ee_pool.tile([128, NS, LQ], fp8)
                for m in range(NS):
                    nc.scalar.activation(E[:, m, :], AL[:, m, :], AF.Exp,
                                         scale=rs[:, m:m + 1], bias=bE4[:])
                t["E"] = E
                return t

            def stage_x2(bi, t):
                """MM2 in fp8 DoubleRow + l2norm; tt = q - wcn (squared later)."""
                qf, cx8, E = t["qf"], t["cx8"], t["E"]
                wc = wc_pool.tile([128, NQ, D], bf16)
                tt = sim_pool.tile([128, NQ, D], bf16)
                ssw2 = sm_pool.tile([128, NQ, 2], f32, tag="ssw2")
                for mq in range(NQ):
                    for n in range(2):
                        sl = slice(512 * n, 512 * (n + 1))
                        psW = psW_pool.tile([128, 512], f32, tag="psW")
                        for k2 in range(NS // 2):
                            nc.tensor.matmul(
                                psW[:],
                                lhsT=E[:, 2 * k2:2 * k2 + 2,
                                       128 * mq:128 * (mq + 1)],
                                rhs=cx8[:, 2 * k2:2 * k2 + 2, sl],
                                start=(k2 == 0), stop=(k2 == NS // 2 - 1),
                                perf_mode=PM.DoubleRow)
                        nc.vector.tensor_copy(wc[:, mq, sl], psW[:])
                        wsq = wrk_pool.tile([128, 512], bf16, tag="wsq")
                        nc.scalar.activation(wsq[:], psW[:], AF.Square,
                                             accum_out=ssw2[:, mq, n:n + 1])
                ssw = sm_pool.tile([128, NQ], f32, tag="ssw")
                nc.vector.tensor_tensor(out=ssw[:], in0=ssw2[:, :, 0],
                                        in1=ssw2[:, :, 1], op=ALU.add)
                g = sm_pool.tile([128, NQ], f32, tag="g")
                rsqrt_dve(g[:], ssw[:], NQ)
                for mq in range(NQ):
                    wn = wrk_pool.tile([128, D], bf16, tag="wn")
                    nc.vector.tensor_scalar_mul(wn[:], wc[:, mq, :],
                                                g[:, mq:mq + 1])
                    nc.vector.tensor_tensor(out=tt[:, mq, :],
                                            in0=qf[:, mq, :],
                                            in1=wn[:], op=ALU.subtract)
                t["tt"] = tt
                return t

            def stage_y(bi, t):
                """simT = (ttT)^2 via PE+ACT, MM3 (+bias row), l2norm, store."""
                tt = t["tt"]
                simT = simT_pool.tile([128, ND, LQ], bf16)
                ss3 = sm_pool.tile([128, NQ], f32, tag="ss3")
                outT = out_pool.tile([128, NQ, S], bf16)
                for tq in range(NQ):
                    pst = psT_pool.tile([128, ND, 128], bf16, tag="psT")
                    for k in range(ND):
                        nc.tensor.transpose(
                            pst[:, k, :], tt[:, tq, 128 * k:128 * (k + 1)],
                            ident[:])
                    nc.scalar.activation(
                        simT[:, :, 128 * tq:128 * (tq + 1)], pst[:],
                        AF.Square)
                psO = psO_pool.tile([128, NQ, S], f32, tag="psO")
                for mq in range(NQ):
                    for k in range(ND):
                        nc.tensor.matmul(
                            psO[:, mq, :],
                            lhsT=simT[:, k, 128 * mq:128 * (mq + 1)],
                            rhs=WT[:, k, :],
                            start=(k == 0), stop=False)
                    nc.tensor.matmul(psO[:, mq, :], lhsT=ones_b[:],
                                     rhs=b_sb[:], start=False, stop=True)
                    junk3 = wrk_pool.tile([128, S], bf16, tag="junk3")
                    nc.scalar.activation(junk3[:], psO[:, mq, :], AF.Square,
                                         accum_out=ss3[:, mq:mq + 1])
                rs3 = sm_pool.tile([128, NQ], f32, tag="rs3")
                rsqrt_dve(rs3[:], ss3[:], NQ)
                for mq in range(NQ):
                    nc.vector.tensor_scalar_mul(outT[:, mq, :],
                                                psO[:, mq, :],
                                                rs3[:, mq:mq + 1])
                t["outT"] = outT

            # ---- software pipeline ----
            # X1b(b+1) (MM1) is issued BEFORE X2(b) (MM2) so the in-order PE
            # queue runs batch b+1's MM1 while batch b's exp chain computes.
            t0 = stage_l(0)
            const_setup()
            t1 = stage_l(1)
            t0 = stage_x1a(0, t0)
            t0 = stage_x1b(0, t0)
            t1 = stage_x1a(1, t1)
            t1 = stage_x1b(1, t1)
            t0 = stage_x2(0, t0)
            t2_ = stage_l(2)
            stage_y(0, t0)
            t2_ = stage_x1a(2, t2_)
            t2_ = stage_x1b(2, t2_)
            t1 = stage_x2(1, t1)
            t3 = stage_l(3)
            stage_y(1, t1)
            t3 = stage_x1a(3, t3)
            t3 = stage_x1b(3, t3)
            t2_ = stage_x2(2, t2_)
            stage_y(2, t2_)
            t3 = stage_x2(3, t3)
            stage_y(3, t3)
            for bi, tt_ in ((0, t0), (1, t1), (2, t2_), (3, t3)):
                nc.gpsimd.dma_start(
                    out_d[bi].rearrange("(t p) s -> p t s", p=128),
                    tt_["outT"][:])

    nc.compile()
    return nc


_NC_CACHE: dict = {}


def kernel(query, context, matrix, W, b, smooth):
    global LAST_EXEC_TIME_NS
    sm = float(smooth)
    nc = _NC_CACHE.get(sm)
    if nc is None:
        nc = _build(sm)
        _NC_CACHE[sm] = nc

    query = np.ascontiguousarray(query, dtype=np.float32)
    context = np.ascontiguousarray(context, dtype=np.float32)
    matrix = np.ascontiguousarray(matrix, dtype=np.float32)
    W = np.ascontiguousarray(W, dtype=np.float32)
    b = np.ascontiguousarray(b, dtype=np.float32)

    in_maps = []
    for c in range(NCORES):
        sl = slice(c * BPC, (c + 1) * BPC)
        in_maps.append({
            "query": query[sl],
            "context": context[sl],
            "matrix": matrix[sl],
            "W": W,
            "b": b,
        })
    res = run_bass_kernel_spmd(nc, in_maps, core_ids=list(range(NCORES)))
    LAST_EXEC_TIME_NS = res.exec_time_ns
    out = np.concatenate([r["out"] for r in res.results], axis=0)
    return out

